# revision 1
# baseline (speedup 1.0000x reference)
"""CLAM-SB MIL forward on 8 Trainium2 NeuronCores (Bass/Tile).

Data-parallel over the bag dimension: core b handles bag b (X[b]: [16384, 1024] f32).
Single pass over X per core:
  - PE transposes X tiles (f32r) -> Xt; h^T = W1^T Xt (PSUM accum over d-chunks)
  - ACT tanh(h + b1) -> th; f columns via th-chunk-stationary matmul vs w2
  - ACT exp(f) -> u grid [128, 128] (col t = row-tile t); w = u * (mask>0)
  - z accumulation: per-tile matmul with w-column stationary, X tile moving
Tail: per-partition top-8 candidates (DVE max8), 64th/65th threshold via
max8/match_replace rounds on a consolidated [2, 1024] row, indirect-DMA gather of
candidate rows, small matmuls vs [Win|Wout], softplus terms, masked sums.
Host combines the per-core scalars into the reference's [10] output.
"""
import numpy as np

import concourse.bacc as bacc
import concourse.bass as bass
import concourse.mybir as mybir
import concourse.tile as tile
from concourse import bass_utils
from concourse.masks import make_identity

f32 = mybir.dt.float32
f32r = mybir.dt.float32r
u32 = mybir.dt.uint32
i32 = mybir.dt.int32
AluOp = mybir.AluOpType
AFT = mybir.ActivationFunctionType
AX = mybir.AxisListType

N, D, A = 16384, 1024, 128
NT = N // 128           # 128 row-tiles
NG = NT // 4            # 32 groups of 4 tiles
NEG = -1.0e30


def build_kernel(stage=99):
    nc = bacc.Bacc("TRN2", target_bir_lowering=False, debug=False, num_devices=8)
    X = nc.dram_tensor("X", [N, D], f32, kind="ExternalInput").ap()
    maskg = nc.dram_tensor("maskg", [128, 128], f32, kind="ExternalInput").ap()
    W1 = nc.dram_tensor("W1", [D, A], f32, kind="ExternalInput").ap()
    b1 = nc.dram_tensor("b1", [128, 1], f32, kind="ExternalInput").ap()
    w2 = nc.dram_tensor("w2", [128, 1], f32, kind="ExternalInput").ap()
    Wd = nc.dram_tensor("Wd", [D, 4], f32, kind="ExternalInput").ap()
    Wc = nc.dram_tensor("Wc", [1, D], f32, kind="ExternalInput").ap()
    cb = nc.dram_tensor("cb", [1, 4], f32, kind="ExternalInput").ap()
    out_vec = nc.dram_tensor("out_vec", [1, 8], f32, kind="ExternalOutput").ap()
    out_cnt = nc.dram_tensor("out_cnt", [2, 2], f32, kind="ExternalOutput").ap()

    with tile.TileContext(nc) as tc:
        consts = tc.alloc_tile_pool(name="consts", bufs=1)
        # identity (f32r) for PE transposes
        ident = consts.tile([128, 128], f32)
        make_identity(nc, ident[:])
        identr = consts.tile([128, 128], f32r)
        nc.vector.tensor_copy(identr[:], ident[:])
        # W1 as [128, 8, 128]: [k, c, a] = W1[128c + k, a]
        W1sb = consts.tile([128, 8, 128], f32)
        nc.sync.dma_start(W1sb[:], W1.rearrange("(c p) a -> p c a", p=128))
        W1r = consts.tile([128, 8, 128], f32r)
        nc.vector.tensor_copy(W1r[:], W1sb[:])
        b1sb = consts.tile([128, 1], f32)
        nc.sync.dma_start(b1sb[:], b1[:])
        w2sb = consts.tile([128, 4], f32)
        nc.vector.memset(w2sb[:], 0.0)
        nc.sync.dma_start(w2sb[:, 0:1], w2[:])
        w2r = consts.tile([128, 4], f32r)
        nc.vector.tensor_copy(w2r[:], w2sb[:])
        Wdsb = consts.tile([128, 8, 4], f32)
        nc.sync.dma_start(Wdsb[:], Wd.rearrange("(c p) k -> p c k", p=128))
        Wdr = consts.tile([128, 8, 4], f32r)
        nc.vector.tensor_copy(Wdr[:], Wdsb[:])
        Wcsb = consts.tile([1, D], f32)
        nc.sync.dma_start(Wcsb[:], Wc[:])
        cbsb = consts.tile([1, 4], f32)
        nc.sync.dma_start(cbsb[:], cb[:])
        masksb = consts.tile([128, 128], f32)
        nc.sync.dma_start(masksb[:], maskg[:])
        mask01 = consts.tile([128, 128], f32)
        nc.vector.tensor_scalar(mask01[:], masksb[:], 0.0, None, op0=AluOp.is_gt)
        iota_p = consts.tile([128, 1], i32)
        nc.gpsimd.iota(iota_p[:], pattern=[[0, 1]], base=0, channel_multiplier=1)
        iota_pf = consts.tile([128, 1], f32)
        nc.vector.tensor_copy(iota_pf[:], iota_p[:])
        onesf = consts.tile([128, 4], f32)
        nc.vector.memset(onesf[:], 1.0)
        onesr = consts.tile([128, 4], f32r)
        nc.vector.tensor_copy(onesr[:], onesf[:])

        # persistent grids
        u_grid = consts.tile([128, 128], f32r)    # exp(f), col t = tile t
        w_grid = consts.tile([128, 128], f32r)    # u * mask01

        # ---- streaming pools (note stack order: z psum first so it outlives others)
        zpool = tc.alloc_tile_pool(name="zpool", bufs=1, space="PSUM")
        z0 = zpool.tile([1, 512], f32)
        z1 = zpool.tile([1, 512], f32)
        xpool = tc.alloc_tile_pool(name="xpool", bufs=12)
        xtgp = tc.alloc_tile_pool(name="xtgp", bufs=3)
        thp = tc.alloc_tile_pool(name="thp", bufs=3)
        ps_xt = tc.alloc_tile_pool(name="ps_xt", bufs=3, space="PSUM")
        ps_h = tc.alloc_tile_pool(name="ps_h", bufs=2, space="PSUM")
        ps_f = tc.alloc_tile_pool(name="ps_f", bufs=1, space="PSUM")

        for g in range(NG):
            xt_g = xtgp.tile([128, 8, 512], f32r, name=f"xtg{g}", tag="xtg")
            xpair = []
            for d2 in range(2):
                x2 = xpool.tile([128, 2, D], f32r, name=f"x{g}_{d2}", tag="x2", bufs=5)
                r0 = 256 * (2 * g + d2)
                nc.gpsimd.dma_start(
                    x2[:], X[r0:r0 + 256, :].rearrange("(a p) d -> p a d", p=128))
                xpair.append(x2)
            xtiles = [xpair[i // 2][:, i % 2] for i in range(4)]
            for t4 in range(4):
                t = 4 * g + t4
                xt_tile = xtiles[t4]
                for h in range(2):
                    ptr = ps_xt.tile([128, 512], f32r, name=f"pxt{t}_{h}", tag="pxt")
                    for i in range(4):
                        c = 4 * h + i
                        nc.tensor.transpose(
                            ptr[:, 128 * i:128 * (i + 1)],
                            xt_tile[:, 128 * c:128 * (c + 1)],
                            identr[:],
                        )
                    # copy psum -> xt_g[:, 4h:4h+4, 128*t4:128*t4+128]
                    dst = xt_g[:, 4 * h:4 * h + 4, 128 * t4:128 * (t4 + 1)]
                    src = ptr.rearrange("p (c q) -> p c q", c=4)
                    if (t + h) % 2 == 0:
                        nc.vector.tensor_copy(dst, src)
                    else:
                        nc.scalar.copy(dst, src)

            # h^T = sum_c W1_c^T Xt_c  -> [a=128, 512 rows]
            ph = ps_h.tile([128, 512], f32, name=f"ph{g}", tag="ph")
            for c in range(8):
                nc.tensor.matmul(ph[:], W1r[:, c, :], xt_g[:, c, :],
                                 start=(c == 0), stop=(c == 7))
            th = thp.tile([128, 512], f32r, name=f"th{g}", tag="th")
            nc.scalar.activation(th[:], ph[:], AFT.Tanh, bias=b1sb[:, :1], scale=1.0)

            # f columns: lhsT = th chunk [K=a, M=128 rows], rhs = w2 -> [128, 1]
            pf = ps_f.tile([128, 16], f32, name=f"pf{g}", tag="pf")
            for t4 in range(4):
                nc.tensor.matmul(pf[:, 4 * t4:4 * t4 + 4],
                                 th[:, 128 * t4:128 * (t4 + 1)], w2r[:],
                                 start=True, stop=True)
            # u = exp(f); w = u * mask01  (f is every 4th column of pf)
            nc.scalar.activation(u_grid[:, 4 * g:4 * g + 4],
                                 pf[:].rearrange("p (t q) -> p t q", q=4)[:, :, 0:1],
                                 AFT.Exp, bias=0.0, scale=1.0)
            nc.vector.tensor_tensor(w_grid[:, 4 * g:4 * g + 4],
                                    u_grid[:, 4 * g:4 * g + 4].bitcast(f32),
                                    mask01[:, 4 * g:4 * g + 4], op=AluOp.mult)

            # z accumulation: per tile, lhsT = w column, rhs = X tile
            for t4 in range(4):
                t = 4 * g + t4
                nc.tensor.matmul(z0[:], w_grid[:, t:t + 1],
                                 xtiles[t4][:, 0:512],
                                 start=(t == 0), stop=(t == NT - 1),
                                 skip_group_check=True)
                nc.tensor.matmul(z1[:], w_grid[:, t:t + 1],
                                 xtiles[t4][:, 512:1024],
                                 start=(t == 0), stop=(t == NT - 1),
                                 skip_group_check=True)

        ps_f.release()
        ps_h.release()
        ps_xt.release()

        # ---------- tail ----------
        tailp = tc.alloc_tile_pool(name="tailp", bufs=1)
        ps_zf = tc.alloc_tile_pool(name="ps_zf", bufs=1, space="PSUM")

        # L = sum(w_grid); z /= L
        Lpart = tailp.tile([128, 1], f32r)
        with nc.allow_low_precision("f32r partial sums feed exact f32 PSUM reduce"):
            nc.vector.tensor_reduce(Lpart[:], w_grid.bitcast(f32)[:], axis=AX.X, op=AluOp.add)
        pL = ps_zf.tile([1, 4], f32)
        nc.tensor.matmul(pL[:], Lpart[:], onesr[:], start=True, stop=True)
        recipL = tailp.tile([1, 1], f32)
        nc.vector.reciprocal(recipL[:], pL[:, 0:1])
        z_sb = tailp.tile([1, D], f32)
        nc.scalar.activation(z_sb[:, 0:512], z0[:], AFT.Copy, bias=0.0, scale=recipL[:, :1])
        nc.scalar.activation(z_sb[:, 512:1024], z1[:], AFT.Copy, bias=0.0, scale=recipL[:, :1])

        if stage < 0:
            nc.sync.dma_start(out_vec[:], z_sb[:, 0:8])
        else:
            outt = tailp.tile([1, 8], f32)
            nc.vector.memset(outt[:], 0.0)
            scr = tailp.tile([1, D], f32)
            nc.vector.tensor_tensor(scr[:], z_sb[:], Wcsb[:], op=AluOp.mult)
            nc.vector.tensor_reduce(outt[:, 0:1], scr[:], axis=AX.X, op=AluOp.add)

            # candidates: top-8 per partition of u (and of -u)
            v8 = tailp.tile([128, 8], f32)
            i8 = tailp.tile([128, 8], u32)
            nc.vector.max(v8[:], u_grid.bitcast(f32)[:])
            nc.vector.max_index(i8[:], v8[:], u_grid.bitcast(f32)[:])
            uneg = tailp.tile([128, 128], f32)
            nc.vector.tensor_scalar(uneg[:], u_grid.bitcast(f32)[:], -1.0, None, op0=AluOp.mult)
            v8b = tailp.tile([128, 8], f32)
            i8b = tailp.tile([128, 8], u32)
            nc.vector.max(v8b[:], uneg[:])
            nc.vector.max_index(i8b[:], v8b[:], uneg[:])

            # global row indices gidx = col_idx * 128 + p
            def to_gidx(i8t, name):
                i8f = tailp.tile([128, 8], f32, name=name + "f")
                nc.vector.tensor_copy(i8f[:], i8t[:])
                gf = tailp.tile([128, 8], f32, name=name + "gf")
                nc.vector.tensor_scalar(gf[:], i8f[:], 128.0, iota_pf[:, :1],
                                        op0=AluOp.mult, op1=AluOp.add)
                gi = tailp.tile([128, 8], u32, name=name + "gi")
                nc.vector.tensor_copy(gi[:], gf[:])
                return gi

            gidx = to_gidx(i8, "gidx_t")
            gidxb = to_gidx(i8b, "gidx_b")

            # consolidate candidate values to [2, 1024] row form (p-major: col = 8p + c)
            cand2 = tailp.tile([2, 1024], f32)
            nc.sync.dma_start(cand2[0:1, :], v8[:])
            nc.sync.dma_start(cand2[1:2, :], v8b[:])
            candB0 = tailp.tile([1, 1024], f32)
            nc.sync.dma_start(candB0[:], v8b[:])

            # threshold: 8 rounds of max8 + match_replace -> 64th; one more max8 -> 65th
            work = tailp.tile([2, 1024], f32)
            nc.vector.tensor_copy(work[:], cand2[:])
            m8 = tailp.tile([2, 8], f32)
            v64 = tailp.tile([2, 1], f32)
            for r in range(8):
                nc.vector.max(m8[:], work[:])
                if r == 7:
                    nc.vector.tensor_copy(v64[:], m8[:, 7:8])
                nc.vector.match_replace(work[:], m8[:], work[:], NEG)
            m8b = tailp.tile([2, 8], f32)
            nc.vector.max(m8b[:], work[:])
            thr2 = tailp.tile([2, 1], f32)
            nc.vector.tensor_scalar(thr2[:], v64[:], m8b[:, 0:1], 0.5,
                                    op0=AluOp.add, op1=AluOp.mult)

            # selection rows + counts (everything on partition 0)
            thrB0 = tailp.tile([1, 1], f32)
            nc.sync.dma_start(thrB0[:], thr2[1:2, :1])
            selT = tailp.tile([1, 1024], f32)
            nc.vector.tensor_scalar(selT[:], cand2[0:1, :], thr2[0:1, :1], None, op0=AluOp.is_gt)
            selB = tailp.tile([1, 1024], f32)
            nc.vector.tensor_scalar(selB[:], candB0[:], thrB0[:, :1], None, op0=AluOp.is_gt)
            cnts = tailp.tile([1, 4], f32)
            nc.vector.tensor_reduce(cnts[:, 0:1], selT[:], axis=AX.X, op=AluOp.add)
            nc.vector.tensor_reduce(cnts[:, 1:2], selB[:], axis=AX.X, op=AluOp.add)
            # 8th-slot hits: p-major layout -> slot c=7 at cols 8p+7 (stride-8 view)
            c8t = tailp.tile([1, 128], f32)
            nc.vector.tensor_copy(c8t[:].rearrange("o (a p) -> o a p", a=1),
                                  selT[:].rearrange("o (p j) -> o j p", p=128)[:, 7:8, :])
            nc.vector.tensor_reduce(cnts[:, 2:3], c8t[:], axis=AX.X, op=AluOp.add)
            c8b = tailp.tile([1, 128], f32)
            nc.vector.tensor_copy(c8b[:].rearrange("o (a p) -> o a p", a=1),
                                  selB[:].rearrange("o (p j) -> o j p", p=128)[:, 7:8, :])
            nc.vector.tensor_reduce(cnts[:, 3:4], c8b[:], axis=AX.X, op=AluOp.add)
            nc.sync.dma_start(out_cnt[:], cnts[:].rearrange("o (a b) -> (o a) b", a=2))

            ps_zf.release()
            zpool.release()

            # gather candidate rows + transpose + arg rows (diff-weight matmuls)

        skip_tail = stage < 1
        if skip_tail and stage >= 0:
            nc.sync.dma_start(out_vec[:], outt[:])
        ps_tail = None
        if not skip_tail:
            ps_tail = tc.alloc_tile_pool(name="ps_tail", bufs=1, space="PSUM")
            arg_ti = ps_tail.tile([1, 1024], f32)   # top, in-class diff (Wd col 0)
            arg_to = ps_tail.tile([1, 1024], f32)   # top, out-class diff (Wd col 2)
            arg_bi = ps_tail.tile([1, 1024], f32)   # bottom, in-class diff (Wd col 1)

            def side_logits(gidx_t, args, side):
                # args: list of (psum_row, wd_col)
                for grp in range(2):
                    xtg_t = xtgp.tile([128, 8, 512], f32r, name=f"xtt{side}{grp}", tag="xtg")
                    for j4 in range(4):
                        j = 4 * grp + j4
                        gt = xpool.tile([128, D], f32r, name=f"g{side}{j}", tag="x", bufs=4)
                        nc.gpsimd.indirect_dma_start(
                            out=gt[:], out_offset=None, in_=X[:],
                            in_offset=bass.IndirectOffsetOnAxis(ap=gidx_t[:, j:j + 1], axis=0))
                        for h in range(2):
                            ptr2 = ps_tail.tile([128, 512], f32r, name=f"pt{side}{j}{h}",
                                                tag="ptail", bufs=2)
                            for i in range(4):
                                c = 4 * h + i
                                nc.tensor.transpose(
                                    ptr2[:, 128 * i:128 * (i + 1)],
                                    gt[:, 128 * c:128 * (c + 1)],
                                    identr[:])
                            dst = xtg_t[:, 4 * h:4 * h + 4, 128 * j4:128 * (j4 + 1)]
                            src = ptr2.rearrange("p (c q) -> p c q", c=4)
                            if (j + h) % 2 == 0:
                                nc.vector.tensor_copy(dst, src)
                            else:
                                nc.scalar.copy(dst, src)
                    for (prow, wcol) in args:
                        for c in range(8):
                            nc.tensor.matmul(prow[:, 512 * grp:512 * (grp + 1)],
                                             Wdr[:, c, wcol:wcol + 1], xtg_t[:, c, :],
                                             start=(c == 0), stop=(c == 7))

            side_logits(gidx, [(arg_ti, 0), (arg_to, 2)], "t")
            side_logits(gidxb, [(arg_bi, 1)], "b")

            # softplus terms and masked sums
            def wsum(argrow, biascol, selr, outslot, name):
                ee = tailp.tile([1, 1024], f32, name=name + "e")
                nc.scalar.activation(ee[:], argrow[:], AFT.Exp,
                                     bias=cbsb[:, biascol:biascol + 1], scale=1.0)
                sp = tailp.tile([1, 1024], f32, name=name + "s")
                nc.scalar.activation(sp[:], ee[:], AFT.Ln, bias=1.0, scale=1.0)
                # sp cols are j-major (128j + p); selr cols are p-major (8p + j):
                # reorder sp to p-major with a strided copy, then flat TTR.
                sp_pm = tailp.tile([1, 1024], f32, name=name + "pm")
                nc.vector.tensor_copy(sp_pm[:].rearrange("o (p j) -> o p j", p=128),
                                      sp[:].rearrange("o (j p) -> o p j", p=128))
                ws = tailp.tile([1, 1024], f32, name=name + "w")
                nc.vector.tensor_tensor(ws[:], sp_pm[:], selr, op=AluOp.mult)
                nc.vector.tensor_reduce(outt[:, outslot:outslot + 1], ws[:],
                                        axis=AX.X, op=AluOp.add)

            wsum(arg_ti, 0, selT[:], 1, "it")   # in-loss, top (y=1)
            wsum(arg_bi, 1, selB[:], 2, "ib")   # in-loss, bottom (y=0)
            wsum(arg_to, 2, selT[:], 3, "ot")   # out-loss, top (y=0)


            nc.sync.dma_start(out_vec[:], outt[:])

        if ps_tail is not None:
            ps_tail.release()
        tailp.release()
        thp.release()
        xtgp.release()
        xpool.release()
        consts.release()

    nc.compile()
    return nc


_NC_CACHE = None


def _get_nc():
    global _NC_CACHE
    if _NC_CACHE is None:
        import os
        _NC_CACHE = build_kernel(int(os.environ.get("KSTAGE", "99")))
    return _NC_CACHE


def make_in_maps(X, mask, labels, W1, b1, w2, b2, Wc, bc, Wi, bi):
    X = np.asarray(X, dtype=np.float32)
    mask = np.asarray(mask, dtype=np.float32)
    labels = np.asarray(labels).astype(np.int64)
    W1 = np.asarray(W1, dtype=np.float32)
    b1v = np.asarray(b1, dtype=np.float32).reshape(128, 1)
    w2v = np.asarray(w2, dtype=np.float32).reshape(128, 1)
    Wc = np.asarray(Wc, dtype=np.float32)
    Wi = np.asarray(Wi, dtype=np.float32)
    bi = np.asarray(bi, dtype=np.float32)
    in_maps = []
    for b in range(8):
        lab = int(labels[b])
        Win, Wout = Wi[lab], Wi[1 - lab]
        Wdm = np.stack([Win[:, 0] - Win[:, 1],
                        Win[:, 1] - Win[:, 0],
                        Wout[:, 1] - Wout[:, 0],
                        np.zeros(D, np.float32)], axis=1)  # [1024, 4]
        bin_, bout = bi[lab], bi[1 - lab]
        cb = np.array([[1.0 + bin_[0] - bin_[1],
                        1.0 + bin_[1] - bin_[0],
                        1.0 + bout[1] - bout[0], 0.0]], dtype=np.float32)
        maskgrid = np.ascontiguousarray(mask[b].reshape(128, 128).T)
        in_maps.append({
            "X": np.ascontiguousarray(X[b]),
            "maskg": maskgrid,
            "W1": W1,
            "b1": b1v,
            "w2": w2v,
            "Wd": np.ascontiguousarray(Wdm),
            "Wc": Wc.reshape(1, D),
            "cb": cb,
        })
    return in_maps


def assemble(results, labels, bc):
    labels = np.asarray(labels).astype(np.float64)
    bag_pred = np.zeros(8, dtype=np.float64)
    inst = 0.0
    for b in range(8):
        ov = results[b]["out_vec"][0].astype(np.float64)
        bag_pred[b] = ov[0] + float(np.asarray(bc).reshape(-1)[0])
        inst += (ov[1] + ov[2]) / 128.0 + ov[3] / 64.0
    crit = np.mean(np.logaddexp(0.0, bag_pred) - bag_pred * labels)
    out = np.concatenate([bag_pred, [crit], [inst]]).astype(np.float32)
    return out


def kernel(X, mask, labels, W1, b1, w2, b2, Wc, bc, Wi, bi):
    nc = _get_nc()
    in_maps = make_in_maps(X, mask, labels, W1, b1, w2, b2, Wc, bc, Wi, bi)
    res = bass_utils.run_bass_kernel_spmd(nc, in_maps, core_ids=list(range(8)))
    return assemble(res.results, labels, bc)



# revision 10
# speedup vs baseline: 485.0385x; 485.0385x over previous
"""CLAM-SB MIL forward on 8 Trainium2 NeuronCores (Bass/Tile).

Data-parallel over the bag dimension: core b handles bag b.

Host prep: X[b] is cast to bf16 and pre-transposed to Xt [1024, 16384]
(d-major), and the per-bag classifier weights are packed into one rhs
matrix Wa = [W1 | Wd0 | Wd1 | Wd2 | Wc] (1024 x 132, bf16), where Wd* are
the logit-difference columns of the label-selected instance classifiers.

Device, single pass over Xt (one matmul family does everything):
  for each 128-row tile t: psum[t] = Xt_chunk(t)^T-stationary @ Wa
    -> [128 rows(n) x 132]: cols 0..127 = h (pre-tanh), 128..130 = instance
       logit diffs, 131 = per-instance bag score c_n = x_n . Wc
  f = w2 . tanh(h + b1) via DVE dot -> u = exp(f) grid [128, 128]
  (col t = row-tile t), w = u * (mask>0); logits+c copied to Lgrid.
No PE transposes, no z matmuls, no tail gather: bag_pred = sum(w*c)/L,
and the instance losses are softplus over the FULL logit grids masked by
the top/bottom-64 selections (thresholds from per-partition top-8
candidates + max8/match_replace rounds, as before).
Host combines the per-core scalars into the reference's [10] output.

build_kernel(rep=K) wraps the whole per-core body in a tc.For_i hardware
loop executing it K times back-to-back in one NEFF - used only for timing
(the slope of wall time in K isolates per-body device time from RPC
dispatch costs). The graded kernel() path uses rep=1.
"""
import numpy as np

import concourse.bacc as bacc
import concourse.bass as bass
import concourse.mybir as mybir
import concourse.tile as tile
from concourse import bass_utils

f32 = mybir.dt.float32
bf16 = mybir.dt.bfloat16
u32 = mybir.dt.uint32
AluOp = mybir.AluOpType
AFT = mybir.ActivationFunctionType
AX = mybir.AxisListType

N, D, A = 16384, 1024, 128
NT = N // 128           # 128 row-tiles
NG = NT // 4            # 32 groups of 4 tiles
SBG = 2                 # groups per DMA superblock (1024 rows)
NSB = NG // SBG         # 16 superblocks
KW = 132                # rhs width: 128 h + 3 logit diffs + 1 bag score
NEG = -1.0e30


def build_kernel(stage=99, rep=1, with_b1=False):
    nc = bacc.Bacc("TRN2", target_bir_lowering=False, debug=False, num_devices=8)
    # Xt is stored in stream order: [sb, p, c, nn] so each superblock DMA
    # reads one contiguous 16 KB block per partition (sequential HBM access).
    Xt = nc.dram_tensor("Xt", [NSB, 128, 8, SBG * 512], bf16,
                        kind="ExternalInput").ap()
    Wa = nc.dram_tensor("Wa", [D, KW], bf16, kind="ExternalInput").ap()
    w2r = nc.dram_tensor("w2r", [128, 8, 128], f32, kind="ExternalInput").ap()
    b1rep = (nc.dram_tensor("b1rep", [128, 128], f32, kind="ExternalInput").ap()
             if with_b1 else None)
    maskg = nc.dram_tensor("maskg", [128, 128], f32, kind="ExternalInput").ap()
    cbr = nc.dram_tensor("cbr", [128, 4], f32, kind="ExternalInput").ap()
    out_vec = nc.dram_tensor("out_vec", [1, 8], f32, kind="ExternalOutput").ap()
    out_cnt = nc.dram_tensor("out_cnt", [2, 2], f32, kind="ExternalOutput").ap()

    with tile.TileContext(nc) as tc:
        consts = tc.alloc_tile_pool(name="consts", bufs=1)
        Wasb = consts.tile([128, 8, KW], bf16)
        nc.sync.dma_start(Wasb[:], Wa.rearrange("(c p) k -> p c k", p=128))
        w2sb = consts.tile([128, 8, 128], f32)
        nc.sync.dma_start(w2sb[:], w2r[:])
        if with_b1:
            b1sb = consts.tile([128, 128], f32)
            nc.sync.dma_start(b1sb[:], b1rep[:])
        masksb = consts.tile([128, 128], f32)
        nc.sync.dma_start(masksb[:], maskg[:])
        mask01 = consts.tile([128, 128], f32)
        nc.vector.tensor_scalar(mask01[:], masksb[:], 0.0, None, op0=AluOp.is_gt)
        ones1 = consts.tile([1, 128], f32)
        nc.vector.memset(ones1[:], 1.0)
        cbsb = consts.tile([128, 4], f32)
        nc.sync.dma_start(cbsb[:], cbr[:])
        onesc = consts.tile([128, 4], f32)
        nc.vector.memset(onesc[:], 1.0)

        # persistent grids: [p, t] = row n = 128*t + p
        u_grid = consts.tile([128, 128], f32)     # exp(f)
        w_grid = consts.tile([128, 128], f32)     # u * mask01
        Lgrid = consts.tile([128, 128, 4], f32)   # logit diffs 0..2, col 3 = c_n

        def emit_body():
            xp = tc.alloc_tile_pool(name="xp", bufs=2)
            thp = tc.alloc_tile_pool(name="thp", bufs=2)
            ps_h = tc.alloc_tile_pool(name="ps_h", bufs=3, space="PSUM")

            for sb in range(NSB):
                xsb = xp.tile([128, 8, SBG * 512], bf16, name=f"x{sb}", tag="x",
                              bufs=3)
                nc.sync.dma_start(xsb[:], Xt[sb])
                if stage == 0:
                    # DMA-only ablation: touch the tile so it isn't dead
                    probe = thp.tile([128, 1], bf16, name=f"pr{sb}", tag="pr")
                    nc.vector.tensor_copy(probe[:], xsb[:, 0, 0:1])
                    continue
                phs = []
                for gi in range(SBG):
                    g = sb * SBG + gi
                    for h2 in range(2):
                        ph = ps_h.tile([128, 2, KW], f32, name=f"ph{g}_{h2}",
                                       tag=f"ph{gi}_{h2}", bufs=2)
                        phs.append(ph)
                        for tt in range(2):
                            off = gi * 512 + (2 * h2 + tt) * 128
                            for c in range(8):
                                nc.tensor.matmul(ph[:, tt, :],
                                                 xsb[:, c, off:off + 128],
                                                 Wasb[:, c, :],
                                                 start=(c == 0), stop=(c == 7))
                        if with_b1:
                            nc.vector.tensor_tensor(
                                ph[:, :, 0:128], ph[:, :, 0:128],
                                b1sb[:].rearrange("p (o a) -> p o a", o=1),
                                op=AluOp.add)
                if stage == 1:
                    probe = thp.tile([128, 1], f32, name=f"pg{sb}", tag="pr1")
                    nc.vector.tensor_copy(probe[:], phs[0][:, 0, 0:1])
                    continue
                # tanh straight from PSUM, then f = sum_a th * w2
                th = thp.tile([128, 8, 128], f32, name=f"th{sb}", tag="th")
                for q in range(4):
                    nc.scalar.activation(th[:, 2 * q:2 * q + 2, :],
                                         phs[q][:, :, 0:128], AFT.Tanh,
                                         bias=0.0, scale=1.0)
                scr = thp.tile([128, 8, 128], f32, name=f"sc{sb}", tag="sc")
                nc.vector.tensor_tensor(scr[:], th[:], w2sb[:], op=AluOp.mult)
                fcol = thp.tile([128, 8], f32, name=f"f{sb}", tag="f")
                nc.vector.tensor_reduce(
                    fcol[:].rearrange("p (f o) -> p f o", o=1),
                    scr[:], axis=AX.X, op=AluOp.add)
                nc.scalar.activation(u_grid[:, 8 * sb:8 * sb + 8], fcol[:],
                                     AFT.Exp, bias=0.0, scale=1.0)
                nc.vector.tensor_tensor(w_grid[:, 8 * sb:8 * sb + 8],
                                        u_grid[:, 8 * sb:8 * sb + 8],
                                        mask01[:, 8 * sb:8 * sb + 8],
                                        op=AluOp.mult)
                # logit diffs (+cb) and bag scores into Lgrid
                for q in range(4):
                    dst = Lgrid[:, 8 * sb + 2 * q:8 * sb + 2 * q + 2, :]
                    if q % 2 == 0:
                        nc.vector.tensor_copy(dst, phs[q][:, :, 128:132])
                    else:
                        nc.scalar.copy(dst, phs[q][:, :, 128:132])

            ps_h.release()

            if stage <= 2:
                tailp = tc.alloc_tile_pool(name="tailp", bufs=1)
                outt = tailp.tile([1, 8], f32)
                nc.vector.memset(outt[:], 0.0)
                nc.sync.dma_start(out_vec[:], outt[:])
                cnts = tailp.tile([2, 2], f32)
                nc.vector.memset(cnts[:], 0.0)
                nc.sync.dma_start(out_cnt[:], cnts[:])
                tailp.release()
                thp.release()
                xp.release()
                return

            # ---------- tail ----------
            tailp = tc.alloc_tile_pool(name="tailp", bufs=1)
            ps_t = tc.alloc_tile_pool(name="ps_t", bufs=1, space="PSUM")

            # L = sum(w_grid); bag dot = sum(w * c)
            S4 = tailp.tile([128, 4], f32)
            nc.vector.tensor_reduce(S4[:, 0:1], w_grid[:], axis=AX.X, op=AluOp.add)
            pL = ps_t.tile([1, 4], f32)
            nc.tensor.matmul(pL[:], S4[:, 0:1], onesc[:], start=True, stop=True)
            recipL = tailp.tile([1, 1], f32)
            nc.vector.reciprocal(recipL[:], pL[:, 0:1])
            cw = tailp.tile([128, 128], f32)
            nc.vector.tensor_tensor(cw[:], w_grid[:], Lgrid[:, :, 3], op=AluOp.mult)

            # candidates: top-8 per partition of u (and of -u)
            v8 = tailp.tile([128, 8], f32)
            nc.vector.max(v8[:], u_grid[:])
            uneg = tailp.tile([128, 128], f32)
            nc.vector.tensor_scalar(uneg[:], u_grid[:], -1.0, None, op0=AluOp.mult)
            v8b = tailp.tile([128, 8], f32)
            nc.vector.max(v8b[:], uneg[:])

            # consolidate candidate values to [2, 1024] rows (p-major: col = 8p+c)
            cand2 = tailp.tile([2, 1024], f32)
            nc.sync.dma_start(cand2[0:1, :], v8[:])
            nc.sync.dma_start(cand2[1:2, :], v8b[:])
            candB0 = tailp.tile([1, 1024], f32)
            nc.sync.dma_start(candB0[:], v8b[:])

            # threshold: 8 rounds of max8 + match_replace -> 64th; one more
            # max8 -> 65th; thr = midpoint
            work = tailp.tile([2, 1024], f32)
            nc.vector.tensor_copy(work[:], cand2[:])
            m8 = tailp.tile([2, 8], f32)
            v64 = tailp.tile([2, 1], f32)
            for r in range(8):
                nc.vector.max(m8[:], work[:])
                if r == 7:
                    nc.vector.tensor_copy(v64[:], m8[:, 7:8])
                nc.vector.match_replace(work[:], m8[:], work[:], NEG)
            m8b = tailp.tile([2, 8], f32)
            nc.vector.max(m8b[:], work[:])
            thr2 = tailp.tile([2, 1], f32)
            nc.vector.tensor_scalar(thr2[:], v64[:], m8b[:, 0:1], 0.5,
                                    op0=AluOp.add, op1=AluOp.mult)

            # candidate-space selections -> counts + 8th-slot guard (out_cnt)
            selTB = tailp.tile([2, 1024], f32)
            nc.vector.tensor_scalar(selTB[:], cand2[:], thr2[:, :1], None,
                                    op0=AluOp.is_gt)
            cnt2 = tailp.tile([2, 2], f32)
            nc.vector.tensor_reduce(cnt2[:, 0:1], selTB[:], axis=AX.X, op=AluOp.add)
            nc.vector.tensor_reduce(
                cnt2[:, 1:2].rearrange("q (a o) -> q a o", a=1),
                selTB[:].rearrange("q (p j) -> q p j", p=128)[:, :, 7:8],
                axis=AX.XY, op=AluOp.add)
            nc.sync.dma_start(out_cnt[:], cnt2[:])

            # broadcast thresholds to all partitions: thrps = ones1^T @ [thrT thrB]
            thrrow = tailp.tile([1, 2], f32)
            nc.sync.dma_start(thrrow[:], thr2[:])
            thrps = ps_t.tile([128, 2], f32)
            nc.tensor.matmul(thrps[:], ones1[:], thrrow[:], start=True, stop=True)
            thrsb = tailp.tile([128, 2], f32)
            nc.scalar.copy(thrsb[:], thrps[:])

            # grid-space selections
            selgT = tailp.tile([128, 128], f32)
            nc.vector.tensor_scalar(selgT[:], u_grid[:], thrsb[:, 0:1], None,
                                    op0=AluOp.is_gt)
            selgB = tailp.tile([128, 128], f32)
            nc.vector.tensor_scalar(selgB[:], uneg[:], thrsb[:, 1:2], None,
                                    op0=AluOp.is_gt)

            # softplus over full logit grids, masked sums
            # slot 1: top sel, diff col 0; slot 2: bottom sel, col 1;
            # slot 3: top sel, col 2
            for slot, (k, selg) in enumerate(
                    [(0, selgT), (1, selgB), (2, selgT)], start=1):
                ee = tailp.tile([128, 128], f32, name=f"ee{k}{slot}")
                nc.scalar.activation(ee[:], Lgrid[:, :, k], AFT.Exp,
                                     bias=cbsb[:, k:k + 1], scale=1.0)
                sp = tailp.tile([128, 128], f32, name=f"sp{k}{slot}")
                nc.scalar.activation(sp[:], ee[:], AFT.Ln, bias=1.0, scale=1.0)
                ws = tailp.tile([128, 128], f32, name=f"ws{k}{slot}")
                nc.vector.tensor_tensor(ws[:], sp[:], selg[:], op=AluOp.mult)
                nc.vector.tensor_reduce(S4[:, slot:slot + 1], ws[:], axis=AX.X,
                                        op=AluOp.add)
            # overwrite S4 col 0 with the bag dot now that cw is ready
            nc.vector.tensor_reduce(S4[:, 0:1], cw[:], axis=AX.X, op=AluOp.add)

            # partition sums: pS[m, j] = sum_p S4[p, m]; slot 0 scaled by 1/L
            pS = ps_t.tile([4, 4], f32)
            nc.tensor.matmul(pS[:], S4[:], onesc[:], start=True, stop=True)
            psb = tailp.tile([4, 1], f32)
            nc.scalar.copy(psb[:], pS[:, 0:1])
            nc.vector.tensor_scalar(psb[0:1, :], psb[0:1, :], recipL[:, :1],
                                    None, op0=AluOp.mult)
            nc.sync.dma_start(out_vec[:, 0:4], psb[:])

            ps_t.release()
            tailp.release()
            thp.release()
            xp.release()

        if rep == 1:
            emit_body()
        else:
            with tc.For_i(0, rep, 1, hint_engines=tuple(mybir.ALL_ENGINES)):
                emit_body()

        consts.release()

    nc.compile()
    return nc


_NC_CACHE = {}


def _get_nc(with_b1=False):
    global _NC_CACHE
    if _NC_CACHE is None:
        _NC_CACHE = {}
    if with_b1 not in _NC_CACHE:
        import os
        _NC_CACHE[with_b1] = build_kernel(
            int(os.environ.get("KSTAGE", "99")),
            rep=int(os.environ.get("KREP", "1")), with_b1=with_b1)
    return _NC_CACHE[with_b1]


def make_in_maps(X, mask, labels, W1, b1, w2, b2, Wc, bc, Wi, bi):
    from ml_dtypes import bfloat16
    X = np.asarray(X, dtype=np.float32)
    mask = np.asarray(mask, dtype=np.float32)
    labels = np.asarray(labels).astype(np.int64)
    W1 = np.asarray(W1, dtype=np.float32)
    b1v = np.asarray(b1, dtype=np.float32).reshape(1, 1, A)
    w2v = np.asarray(w2, dtype=np.float32).reshape(1, 1, A)
    Wc = np.asarray(Wc, dtype=np.float32).reshape(D, 1)
    Wi = np.asarray(Wi, dtype=np.float32)
    bi = np.asarray(bi, dtype=np.float32)
    w2r = np.ascontiguousarray(np.broadcast_to(w2v, (128, 8, A)))
    b1f = np.asarray(b1, dtype=np.float32).reshape(1, A)
    with_b1 = bool(np.any(b1f))
    b1rep = np.ascontiguousarray(np.broadcast_to(b1f, (128, A)))
    in_maps = []
    for b in range(8):
        lab = int(labels[b])
        Win, Wout = Wi[lab], Wi[1 - lab]
        Wd3 = np.stack([Win[:, 0] - Win[:, 1],
                        Win[:, 1] - Win[:, 0],
                        Wout[:, 1] - Wout[:, 0]], axis=1)  # [1024, 3]
        Wa = np.concatenate([W1, Wd3, Wc], axis=1).astype(bfloat16)  # [1024, 132]
        bin_, bout = bi[lab], bi[1 - lab]
        cb = np.array([1.0 + bin_[0] - bin_[1],
                       1.0 + bin_[1] - bin_[0],
                       1.0 + bout[1] - bout[0], 0.0], dtype=np.float32)
        cbrep = np.ascontiguousarray(np.broadcast_to(cb.reshape(1, 4), (128, 4)))
        maskgrid = np.ascontiguousarray(mask[b].reshape(128, 128).T)
        Xtb = X[b].astype(bfloat16).T  # [1024(d), 16384(n)] view
        # stream-order layout [sb, p, c, nn]: d = 128*c + p, n = SBG*512*sb + nn
        Xts = np.ascontiguousarray(
            Xtb.reshape(8, 128, 32 // SBG, SBG * 512).transpose(2, 1, 0, 3))
        in_maps.append({
            "Xt": Xts,
            "Wa": np.ascontiguousarray(Wa),
            "w2r": w2r,
            "maskg": maskgrid,
            "cbr": cbrep,
        })
        if with_b1:
            in_maps[-1]["b1rep"] = b1rep
    return in_maps


def assemble(results, labels, bc):
    labels = np.asarray(labels).astype(np.float64)
    bag_pred = np.zeros(8, dtype=np.float64)
    inst = 0.0
    for b in range(8):
        ov = results[b]["out_vec"][0].astype(np.float64)
        bag_pred[b] = ov[0] + float(np.asarray(bc).reshape(-1)[0])
        inst += (ov[1] + ov[2]) / 128.0 + ov[3] / 64.0
    crit = np.mean(np.logaddexp(0.0, bag_pred) - bag_pred * labels)
    out = np.concatenate([bag_pred, [crit], [inst]]).astype(np.float32)
    return out


def kernel(X, mask, labels, W1, b1, w2, b2, Wc, bc, Wi, bi):
    nc = _get_nc(with_b1=bool(np.any(np.asarray(b1))))
    in_maps = make_in_maps(X, mask, labels, W1, b1, w2, b2, Wc, bc, Wi, bi)
    res = bass_utils.run_bass_kernel_spmd(nc, in_maps, core_ids=list(range(8)))
    return assemble(res.results, labels, bc)
